# revision 6
# baseline (speedup 1.0000x reference)
"""Causal self-attention on 8 TRN2 NeuronCores (Bass/Tile, SPMD).

Problem: B=4, T=2048, C=1024, H=16, D=64, fp32 in/out.

Sharding: core i = (batch b=i//2, parity p=i%2). Each core computes ALL 16
heads for its interleaved quarter of query positions: 256-wide q-chunks
{0,3,4,7} (parity 0) or {1,2,5,6} (parity 1) of batch b, slot-sorted by
causal prefix so both parities' slots pad to extents {4,8,12,16} key-tiles
of 128 -> every core runs the IDENTICAL instruction stream (SPMD); the
causal mask is host-supplied data. No inter-core communication.

v2 vs baseline (668us):
 - bf16 for all matmul operands (sim rel-err 0.53% vs 2e-2 gate); halves
   DMA + SBUF, keeps Q^T and O^T resident (no DRAM roundtrips).
 - Causal mask folded into PSUM *before* exp as an additive (0/-30)
   identity-matmul accumulate on TensorE (start of the S accumulation
   group) instead of ~256 DVE multiplies after exp.
 - One x^T pass feeds both K^T and V projections.
 - K/V projection slab s is emitted right before attention slot s
   (EXT[s] = 4(s+1) key-tiles = exactly slabs 0..s), so ScalarE exp
   overlaps projection matmuls and the PE never idles long enough to
   re-throttle (HAM).
 - Output projection reads O^T straight from SBUF.
"""
import os
import numpy as np
import ml_dtypes

import concourse.bacc as bacc
import concourse.mybir as mybir
import concourse.tile as tile
from concourse.bass_utils import run_bass_kernel_spmd

B, T, C, H, D = 4, 2048, 1024, 16, 64
QC = 256                      # q-chunk width
NSLOT = 4                     # q-chunks per core
OWN = [[0, 3, 4, 7], [1, 2, 5, 6]]   # global q-chunk ids per parity, slot order
EXT = [4, 8, 12, 16]          # padded key-tile (128) extent per slot
F32 = mybir.dt.float32
BF16 = mybir.dt.bfloat16
VA_W = H * (D + 1)            # 1040: V_aug cols = 16 heads x (64 | ones)
NEG = -30.0                   # additive mask for causally-forbidden keys

_cache = {}


def _build():
    nc = bacc.Bacc("TRN2", target_bir_lowering=False, debug=False,
                   enable_asserts=False, num_devices=8)

    def din(name, shape, dt=BF16):
        return nc.dram_tensor(name, list(shape), dt, kind="ExternalInput").ap()

    xt_d = din("xt", (C, T))            # x[b].T
    xq_d = din("xq", (C, NSLOT * QC))   # own q columns of x[b].T
    wq_d = din("wq", (C, C))            # pre-scaled by 1/8
    wk_d = din("wk", (C, C))
    wv_d = din("wv", (C, C))
    wp_d = din("wp", (C, C))
    bq_d = din("bq", (8, 128, 1), F32)  # pre-scaled by 1/8
    bk_d = din("bk", (8, 128, 1), F32)
    bpeb_d = din("bpeb", (128, C), F32)  # bproj_eff broadcast to 128 partitions
    mk_d = din("masks", (NSLOT, 4, 128, QC))  # additive 0/-30
    id_d = din("id128", (128, 128))
    y_d = nc.dram_tensor("y", [NSLOT * QC, C], F32, kind="ExternalOutput").ap()

    bypass = mybir.AluOpType.bypass
    mult = mybir.AluOpType.mult
    add = mybir.AluOpType.add
    EXP = mybir.ActivationFunctionType.Exp

    with tile.TileContext(nc) as tc:
        # ---------------- persistent tiles ------------------------------
        pers = tc.alloc_tile_pool(name="pers", bufs=1)
        KT = [[pers.tile([128, 512], BF16, name=f"kt{j}_{sl}", tag=f"kt{j}_{sl}")
               for sl in range(4)] for j in range(8)]
        QT = [[pers.tile([128, 512], BF16, name=f"qt{j}_{sl}", tag=f"qt{j}_{sl}")
               for sl in range(2)] for j in range(8)]
        VA = [pers.tile([128, VA_W], BF16, name=f"va{g}", tag=f"va{g}")
              for g in range(16)]
        OT = [[pers.tile([128, QC], BF16, name=f"ot{j}_{s}", tag=f"ot{j}_{s}")
               for s in range(NSLOT)] for j in range(8)]
        MK = [[pers.tile([128, QC], BF16, name=f"mk{s}{mi}", tag=f"mk{s}{mi}")
               for mi in range(4)] for s in range(NSLOT)]
        ID = pers.tile([128, 128], BF16, name="id128", tag="id128")
        nc.sync.dma_start(out=ID[:], in_=id_d)
        for s in range(NSLOT):
            for mi in range(4):
                nc.sync.dma_start(out=MK[s][mi][:], in_=mk_d[s, mi])
        ones16 = pers.tile([128, H], BF16, name="ones16", tag="ones16")
        nc.vector.memset(ones16[:], 1.0)
        ones16_3d = ones16[:].unsqueeze(2)
        for g in range(16):
            dst1 = VA[g][:].rearrange("p (h d) -> p h d", d=D + 1)[:, :, D:D + 1]
            nc.vector.tensor_copy(out=dst1, in_=ones16_3d)

        with tc.tile_pool(name="kvw", bufs=1) as kvw, \
             tc.tile_pool(name="pw", bufs=1) as pw:
            # prefetch K/V/proj weights early so phase transitions have no DMA gap
            wkt = [kvw.tile([128, C], BF16, name=f"wk{c}", tag=f"wk{c}") for c in range(8)]
            wvt = [kvw.tile([128, C], BF16, name=f"wv{c}", tag=f"wv{c}") for c in range(8)]
            bks = [kvw.tile([128, 1], F32, name=f"bk{j}", tag=f"bk{j}") for j in range(8)]
            wpt = [pw.tile([128, C], BF16, name=f"wp{c}", tag=f"wp{c}") for c in range(8)]
            bpeb = pw.tile([128, C], F32, name="bpeb", tag="bpeb")
            for c in range(8):
                nc.sync.dma_start(out=wkt[c][:], in_=wk_d[128*c:128*(c+1), :])
                nc.sync.dma_start(out=wvt[c][:], in_=wv_d[128*c:128*(c+1), :])
                nc.sync.dma_start(out=bks[c][:], in_=bk_d[c])
                nc.sync.dma_start(out=wpt[c][:], in_=wp_d[128*c:128*(c+1), :])
            nc.sync.dma_start(out=bpeb[:], in_=bpeb_d[:])

            # ---------------- Q phase (dense, 8 PSUM banks) --------------
            with tc.tile_pool(name="qw", bufs=1) as qw, \
                 tc.tile_pool(name="qx", bufs=1) as qx, \
                 tc.tile_pool(name="qps", bufs=1, space="PSUM") as qps:
                wqt = [qw.tile([128, C], BF16, name=f"wq{c}", tag=f"wq{c}") for c in range(8)]
                bqs = [qw.tile([128, 1], F32, name=f"bq{j}", tag=f"bq{j}") for j in range(8)]
                for c in range(8):
                    nc.sync.dma_start(out=wqt[c][:], in_=wq_d[128*c:128*(c+1), :])
                    nc.sync.dma_start(out=bqs[c][:], in_=bq_d[c])
                for slab in range(2):
                    xqs = []
                    for c in range(8):
                        t = qx.tile([128, 512], BF16, name=f"xq{c}", tag=f"xq{c}")
                        nc.sync.dma_start(out=t[:], in_=xq_d[128*c:128*(c+1), 512*slab:512*(slab+1)])
                        xqs.append(t)
                    pq = [qps.tile([128, 512], F32, name=f"q{j}", tag=f"q{j}") for j in range(8)]
                    for c in range(8):
                        for j in range(8):
                            nc.tensor.matmul(out=pq[j][:], lhsT=wqt[c][:, 128*j:128*(j+1)],
                                             rhs=xqs[c][:], start=(c == 0), stop=(c == 7))
                    for j in range(8):
                        nc.vector.tensor_scalar_add(out=QT[j][slab][:], in0=pq[j][:],
                                                    scalar1=bqs[j][:])

            # ------------- K/V slabs + attention + projection ------------
            with tc.tile_pool(name="xtp", bufs=2) as xtp, \
                 tc.tile_pool(name="ptp", bufs=2) as ptp, \
                 tc.tile_pool(name="smp", bufs=1) as smp, \
                 tc.tile_pool(name="yp", bufs=2) as yp, \
                 tc.tile_pool(name="kvps", bufs=1, space="PSUM") as kvps, \
                 tc.tile_pool(name="aps", bufs=1, space="PSUM") as aps:

                def load_xt(slab):
                    xts = []
                    for c in range(8):
                        t = xtp.tile([128, 512], BF16, name=f"xt{c}", tag=f"xt{c}")
                        nc.sync.dma_start(out=t[:], in_=xt_d[128*c:128*(c+1), 512*slab:512*(slab+1)])
                        xts.append(t)
                    return xts

                def k_wave(xts, slab, w):
                    pk = [kvps.tile([128, 512], F32, name=f"kv{i}", tag=f"kv{i}") for i in range(2)]
                    for c in range(8):
                        for i in range(2):
                            j = 2 * w + i
                            nc.tensor.matmul(out=pk[i][:], lhsT=wkt[c][:, 128*j:128*(j+1)],
                                             rhs=xts[c][:], start=(c == 0), stop=(c == 7))
                    for i in range(2):
                        j = 2 * w + i
                        nc.vector.tensor_scalar_add(out=KT[j][slab][:], in0=pk[i][:],
                                                    scalar1=bks[j][:])

                def v_wave(xts, slab, tt):
                    g = 4 * slab + tt
                    pv = [kvps.tile([128, 512], F32, name=f"kv{i}", tag=f"kv{i}") for i in range(2)]
                    for c in range(8):
                        for jc in range(2):
                            nc.tensor.matmul(out=pv[jc][:],
                                             lhsT=xts[c][:, 128*tt:128*(tt+1)],
                                             rhs=wvt[c][:, 512*jc:512*(jc+1)],
                                             start=(c == 0), stop=(c == 7))
                    for jc in range(2):
                        dst = VA[g][:, 520*jc:520*(jc+1)].rearrange("p (h d) -> p h d", d=D+1)[:, :, 0:D]
                        src = pv[jc][:].rearrange("p (h d) -> p h d", d=D)
                        nc.vector.tensor_copy(out=dst, in_=src)

                def attn(s, j):
                    E = EXT[s]
                    o2 = aps.tile([65, 512], F32, name="o2", tag="o2", bufs=2)
                    for g in range(E // 2):
                        masked = (2 * g) >= E - 4
                        ss = aps.tile([128, 1024], F32, name="ss", tag="ss", bufs=2)
                        for u in range(2):
                            m = 2 * g + u
                            sl, mm = m // 4, m % 4
                            if masked:
                                mi = m - (E - 4)
                                for h in range(2):
                                    nc.tensor.matmul(out=ss[:, 512*h+QC*u:512*h+QC*(u+1)],
                                                     lhsT=ID[:], rhs=MK[s][mi][:],
                                                     start=True, stop=False,
                                                     skip_group_check=True)
                            for h in range(2):
                                nc.tensor.matmul(
                                    out=ss[:, 512*h+QC*u:512*h+QC*(u+1)],
                                    lhsT=KT[j][sl][64*h:64*(h+1), 128*mm:128*(mm+1)],
                                    rhs=QT[j][s // 2][64*h:64*(h+1), QC*(s % 2):QC*(s % 2 + 1)],
                                    tile_position=(64 * h, 0),
                                    start=(not masked), stop=True,
                                    skip_group_check=masked)
                        pt = ptp.tile([128, 1024], BF16, name="pt", tag="pt")
                        nc.scalar.activation(out=pt[:], in_=ss[:], func=EXP)
                        for u in range(2):
                            m = 2 * g + u
                            for h in range(2):
                                nc.tensor.matmul(out=o2[:, QC*h:QC*(h+1)],
                                                 lhsT=VA[m][:, 65*(2*j+h):65*(2*j+h)+65],
                                                 rhs=pt[:, 512*h+QC*u:512*h+QC*(u+1)],
                                                 start=(m == 0 and h == 0),
                                                 stop=(m == E - 1),
                                                 skip_group_check=True)
                    lsb = smp.tile([1, 512], F32, name="lsb", tag="lsb")
                    nc.vector.tensor_copy(out=lsb[:], in_=o2[64:65, :])
                    rsb = smp.tile([1, 512], F32, name="rsb", tag="rsb")
                    nc.vector.reciprocal_approx_fast(rsb[:], lsb[:])
                    rbb = smp.tile([64, 512], F32, name="rbb", tag="rbb")
                    nc.gpsimd.partition_broadcast(rbb[:], rsb[:])
                    for h in range(2):
                        nc.vector.scalar_tensor_tensor(
                            out=OT[j][s][64*h:64*(h+1), :], in0=o2[0:64, QC*h:QC*(h+1)],
                            scalar=0.0, in1=rbb[:, QC*h:QC*(h+1)],
                            op0=bypass, op1=mult)

                def proj(gi):
                    ti, jc = gi // 2, gi % 2
                    s, half = ti // 2, ti % 2
                    py = kvps.tile([128, 512], F32, name=f"kv{gi % 2}", tag=f"kv{gi % 2}")
                    for c in range(8):
                        nc.tensor.matmul(out=py[:],
                                         lhsT=OT[c][s][:, 128*half:128*(half+1)],
                                         rhs=wpt[c][:, 512*jc:512*(jc+1)],
                                         start=(c == 0), stop=(c == 7))
                    ysb = yp.tile([128, 512], F32, name="ysb", tag="ysb")
                    nc.vector.scalar_tensor_tensor(out=ysb[:], in0=py[:], scalar=0.0,
                                                   in1=bpeb[:, 512*jc:512*(jc+1)],
                                                   op0=bypass, op1=add)
                    nc.sync.dma_start(out=y_d[128*ti:128*(ti+1), 512*jc:512*(jc+1)], in_=ysb[:])

                # slab 0 dense, then slots with slab s+1 / proj interleaved
                xts = load_xt(0)
                for w in range(4):
                    k_wave(xts, 0, w)
                for tt in range(4):
                    v_wave(xts, 0, tt)
                for s in range(NSLOT):
                    if s < 3:
                        nxts = load_xt(s + 1)
                    for j in range(8):
                        attn(s, j)
                        if s < 3:
                            if j < 4:
                                k_wave(nxts, s + 1, j)
                            else:
                                v_wave(nxts, s + 1, j - 4)
                        else:
                            proj(j)
                            if j + 8 < 12:
                                proj(j + 8)
                for gi in range(12, 16):
                    proj(gi)
        pers.release()

    nc.compile()
    return nc


def _get_nc():
    if "nc" not in _cache:
        _cache["nc"] = _build()
    return _cache["nc"]


def _host_prep(x, Wqkv, bqkv, Wproj, bproj):
    bf = ml_dtypes.bfloat16
    x = np.ascontiguousarray(np.asarray(x, dtype=np.float32))
    Wqkv = np.asarray(Wqkv, dtype=np.float32)
    bqkv = np.asarray(bqkv, dtype=np.float32)
    Wproj = np.ascontiguousarray(np.asarray(Wproj, dtype=np.float32))
    bproj = np.asarray(bproj, dtype=np.float32)

    wq = np.ascontiguousarray(Wqkv[:, :C] * np.float32(0.125)).astype(bf)
    wk = np.ascontiguousarray(Wqkv[:, C:2*C]).astype(bf)
    wv = np.ascontiguousarray(Wqkv[:, 2*C:]).astype(bf)
    wp = Wproj.astype(bf)
    bq8 = (bqkv[:C] * np.float32(0.125)).reshape(8, 128, 1).copy()
    bk8 = bqkv[C:2*C].reshape(8, 128, 1).copy()
    bv = bqkv[2*C:]
    bpe = (bproj.astype(np.float64) + bv.astype(np.float64) @ Wproj.astype(np.float64)).astype(np.float32)
    bpeb = np.ascontiguousarray(np.broadcast_to(bpe, (128, C)))
    id128 = np.eye(128, dtype=bf)

    pidx = np.arange(128)[:, None]
    fidx = np.arange(QC)[None, :]
    masks = []
    for par in range(2):
        mk = np.zeros((NSLOT, 4, 128, QC), dtype=np.float32)
        for s, cchunk in enumerate(OWN[par]):
            for mi in range(4):
                g = EXT[s] - 4 + mi
                mk[s, mi] = np.where((128*g + pidx) <= (QC*cchunk + fidx), 0.0, NEG)
        masks.append(mk.astype(bf))

    in_maps = []
    for core in range(8):
        b, par = core // 2, core % 2
        xt = np.ascontiguousarray(x[b].T)
        xq = np.ascontiguousarray(
            np.concatenate([xt[:, QC*c:QC*(c+1)] for c in OWN[par]], axis=1)).astype(bf)
        in_maps.append(dict(xt=xt.astype(bf), xq=xq, wq=wq, wk=wk, wv=wv, wp=wp,
                            bq=bq8, bk=bk8, bpeb=bpeb, masks=masks[par],
                            id128=id128))
    return in_maps


def kernel(x, Wqkv, bqkv, Wproj, bproj):
    nc = _get_nc()
    in_maps = _host_prep(x, Wqkv, bqkv, Wproj, bproj)
    trace = bool(os.environ.get("BASS_TRACE"))
    res = run_bass_kernel_spmd(nc, in_maps, list(range(8)), trace=trace)
    _cache["last_exec_time_ns"] = res.exec_time_ns
    _cache["last_res"] = res
    out = np.empty((B, T, C), dtype=np.float32)
    for core in range(8):
        b, par = core // 2, core % 2
        y = res.results[core]["y"]
        for s, cchunk in enumerate(OWN[par]):
            out[b, QC*cchunk:QC*(cchunk+1)] = y[QC*s:QC*(s+1)]
    return out


# revision 8
# speedup vs baseline: 1.0704x; 1.0704x over previous
"""Causal self-attention on 8 TRN2 NeuronCores (Bass/Tile, SPMD).

Problem: B=4, T=2048, C=1024, H=16, D=64, fp32 in/out.

Sharding: core i = (batch b=i//2, parity p=i%2). Each core computes ALL 16
heads for its interleaved quarter of query positions: 256-wide q-chunks
{0,3,4,7} (parity 0) or {1,2,5,6} (parity 1) of batch b, slot-sorted by
causal prefix so both parities' slots pad to extents {4,8,12,16} key-tiles
of 128 -> every core runs the IDENTICAL instruction stream (SPMD); the
causal mask is host-supplied data. No inter-core communication.

v2 vs baseline (668us):
 - bf16 for all matmul operands (sim rel-err 0.53% vs 2e-2 gate); halves
   DMA + SBUF, keeps Q^T and O^T resident (no DRAM roundtrips).
 - Causal mask folded into PSUM *before* exp as an additive (0/-30)
   identity-matmul accumulate on TensorE (start of the S accumulation
   group) instead of ~256 DVE multiplies after exp.
 - One x^T pass feeds both K^T and V projections.
 - K/V projection slab s is emitted right before attention slot s
   (EXT[s] = 4(s+1) key-tiles = exactly slabs 0..s), so ScalarE exp
   overlaps projection matmuls and the PE never idles long enough to
   re-throttle (HAM).
 - Output projection reads O^T straight from SBUF.
"""
import os
import numpy as np
import ml_dtypes

import concourse.bacc as bacc
import concourse.mybir as mybir
import concourse.tile as tile
from concourse.bass_utils import run_bass_kernel_spmd

B, T, C, H, D = 4, 2048, 1024, 16, 64
QC = 256                      # q-chunk width
NSLOT = 4                     # q-chunks per core
OWN = [[0, 3, 4, 7], [1, 2, 5, 6]]   # global q-chunk ids per parity, slot order
EXT = [4, 8, 12, 16]          # padded key-tile (128) extent per slot
F32 = mybir.dt.float32
BF16 = mybir.dt.bfloat16
VA_W = H * (D + 1)            # 1040: V_aug cols = 16 heads x (64 | ones)
NEG = -30.0                   # additive mask for causally-forbidden keys

_cache = {}


def _build():
    nc = bacc.Bacc("TRN2", target_bir_lowering=False, debug=False,
                   enable_asserts=False, num_devices=8)

    def din(name, shape, dt=BF16):
        return nc.dram_tensor(name, list(shape), dt, kind="ExternalInput").ap()

    xt_d = din("xt", (C, T))            # x[b].T
    xq_d = din("xq", (C, NSLOT * QC))   # own q columns of x[b].T
    wq_d = din("wq", (C, C))            # pre-scaled by 1/8
    wk_d = din("wk", (C, C))
    wv_d = din("wv", (C, C))
    wp_d = din("wp", (C, C))
    bq_d = din("bq", (8, 128, 1), F32)  # pre-scaled by 1/8
    bk_d = din("bk", (8, 128, 1), F32)
    bpeb_d = din("bpeb", (128, C), F32)  # bproj_eff broadcast to 128 partitions
    mk_d = din("masks", (NSLOT, 4, 128, QC))  # additive 0/-30
    id_d = din("id128", (128, 128))
    y_d = nc.dram_tensor("y", [NSLOT * QC, C], F32, kind="ExternalOutput").ap()

    bypass = mybir.AluOpType.bypass
    mult = mybir.AluOpType.mult
    add = mybir.AluOpType.add
    EXP = mybir.ActivationFunctionType.Exp

    with tile.TileContext(nc) as tc:
        # ---------------- persistent tiles ------------------------------
        pers = tc.alloc_tile_pool(name="pers", bufs=1)
        KT = [[pers.tile([128, 512], BF16, name=f"kt{j}_{sl}", tag=f"kt{j}_{sl}")
               for sl in range(4)] for j in range(8)]
        QT = [[pers.tile([128, 512], BF16, name=f"qt{j}_{sl}", tag=f"qt{j}_{sl}")
               for sl in range(2)] for j in range(8)]
        VA = [pers.tile([128, VA_W], BF16, name=f"va{g}", tag=f"va{g}")
              for g in range(16)]
        OT = [[pers.tile([128, QC], BF16, name=f"ot{j}_{s}", tag=f"ot{j}_{s}")
               for s in range(NSLOT)] for j in range(8)]
        MK = [[pers.tile([128, QC], BF16, name=f"mk{s}{mi}", tag=f"mk{s}{mi}")
               for mi in range(4)] for s in range(NSLOT)]
        ID = pers.tile([128, 128], BF16, name="id128", tag="id128")
        nc.sync.dma_start(out=ID[:], in_=id_d)
        for s in range(NSLOT):
            for mi in range(4):
                nc.sync.dma_start(out=MK[s][mi][:], in_=mk_d[s, mi])
        ones16 = pers.tile([128, H], BF16, name="ones16", tag="ones16")
        nc.vector.memset(ones16[:], 1.0)
        ones16_3d = ones16[:].unsqueeze(2)
        for g in range(16):
            dst1 = VA[g][:].rearrange("p (h d) -> p h d", d=D + 1)[:, :, D:D + 1]
            nc.vector.tensor_copy(out=dst1, in_=ones16_3d)

        with tc.tile_pool(name="kvw", bufs=1) as kvw:
            wkt = [kvw.tile([128, C], BF16, name=f"wk{c}", tag=f"wk{c}") for c in range(8)]
            wvt = [kvw.tile([128, C], BF16, name=f"wv{c}", tag=f"wv{c}") for c in range(8)]
            bks = [kvw.tile([128, 1], F32, name=f"bk{j}", tag=f"bk{j}") for j in range(8)]

            # ---- Q phase (dense, 8 PSUM banks); kv-weight DMAs queued behind ----
            with tc.tile_pool(name="qw", bufs=1) as qw, \
                 tc.tile_pool(name="qx", bufs=1) as qx, \
                 tc.tile_pool(name="qps", bufs=1, space="PSUM") as qps:
                wqt = [qw.tile([128, C], BF16, name=f"wq{c}", tag=f"wq{c}") for c in range(8)]
                bqs = [qw.tile([128, 1], F32, name=f"bq{j}", tag=f"bq{j}") for j in range(8)]
                for c in range(8):
                    nc.sync.dma_start(out=wqt[c][:], in_=wq_d[128*c:128*(c+1), :])
                    nc.sync.dma_start(out=bqs[c][:], in_=bq_d[c])
                xq_t = [[None] * 8 for _ in range(2)]
                for slab in range(2):
                    for c in range(8):
                        t = qx.tile([128, 512], BF16, name=f"xq{c}", tag=f"xq{c}")
                        nc.sync.dma_start(out=t[:], in_=xq_d[128*c:128*(c+1), 512*slab:512*(slab+1)])
                        xq_t[slab][c] = t
                    if slab == 0:
                        for c in range(8):
                            nc.sync.dma_start(out=wkt[c][:], in_=wk_d[128*c:128*(c+1), :])
                            nc.sync.dma_start(out=wvt[c][:], in_=wv_d[128*c:128*(c+1), :])
                            nc.sync.dma_start(out=bks[c][:], in_=bk_d[c])
                for slab in range(2):
                    pq = [qps.tile([128, 512], F32, name=f"q{j}", tag=f"q{j}") for j in range(8)]
                    for c in range(8):
                        for j in range(8):
                            nc.tensor.matmul(out=pq[j][:], lhsT=wqt[c][:, 128*j:128*(j+1)],
                                             rhs=xq_t[slab][c][:], start=(c == 0), stop=(c == 7))
                    for j in range(8):
                        nc.vector.tensor_scalar_add(out=QT[j][slab][:], in0=pq[j][:],
                                                    scalar1=bqs[j][:])

            with tc.tile_pool(name="pw", bufs=1) as pw, \
                 tc.tile_pool(name="xtp", bufs=2) as xtp, \
                 tc.tile_pool(name="ptp", bufs=2) as ptp, \
                 tc.tile_pool(name="smp", bufs=1) as smp, \
                 tc.tile_pool(name="yp", bufs=2) as yp, \
                 tc.tile_pool(name="kvps", bufs=1, space="PSUM") as kvps, \
                 tc.tile_pool(name="aps", bufs=1, space="PSUM") as aps:
                wpt = [pw.tile([128, C], BF16, name=f"wp{c}", tag=f"wp{c}") for c in range(8)]
                bpeb = pw.tile([128, C], F32, name="bpeb", tag="bpeb")

                def load_xt(slab):
                    xts = []
                    for c in range(8):
                        t = xtp.tile([128, 512], BF16, name=f"xt{c}", tag=f"xt{c}")
                        nc.sync.dma_start(out=t[:], in_=xt_d[128*c:128*(c+1), 512*slab:512*(slab+1)])
                        xts.append(t)
                    return xts

                xts0 = load_xt(0)
                nc.sync.dma_start(out=ID[:], in_=id_d)
                for s in range(NSLOT):
                    for mi in range(4):
                        nc.sync.dma_start(out=MK[s][mi][:], in_=mk_d[s, mi])
                for c in range(8):
                    nc.sync.dma_start(out=wpt[c][:], in_=wp_d[128*c:128*(c+1), :])
                nc.sync.dma_start(out=bpeb[:], in_=bpeb_d[:])

                # ---- filler step machinery ----
                def slab_steps(xts, slab):
                    """Fine-grained K/V wave steps: each step emits ~2 matmuls."""
                    steps = []
                    for w in range(4):
                        cell = {}
                        def alloc(cell=cell):
                            cell["pk"] = [kvps.tile([128, 512], F32, name=f"kv{i}", tag=f"kv{i}")
                                          for i in range(2)]
                        def kstep(c, w=w, cell=cell):
                            for i in range(2):
                                j = 2 * w + i
                                nc.tensor.matmul(out=cell["pk"][i][:],
                                                 lhsT=wkt[c][:, 128*j:128*(j+1)],
                                                 rhs=xts[c][:], start=(c == 0), stop=(c == 7))
                        def kevac(w=w, cell=cell):
                            for i in range(2):
                                j = 2 * w + i
                                nc.vector.tensor_scalar_add(out=KT[j][slab][:],
                                                            in0=cell["pk"][i][:],
                                                            scalar1=bks[j][:])
                        for c in range(8):
                            if c == 0:
                                steps.append(lambda c=c, a=alloc, k=kstep: (a(), k(c)))
                            else:
                                steps.append(lambda c=c, k=kstep: k(c))
                        steps.append(kevac)
                    for tt in range(4):
                        g = 4 * slab + tt
                        cell = {}
                        def valloc(cell=cell):
                            cell["pv"] = [kvps.tile([128, 512], F32, name=f"kv{i}", tag=f"kv{i}")
                                          for i in range(2)]
                        def vstep(c, tt=tt, cell=cell):
                            for jc in range(2):
                                nc.tensor.matmul(out=cell["pv"][jc][:],
                                                 lhsT=xts[c][:, 128*tt:128*(tt+1)],
                                                 rhs=wvt[c][:, 512*jc:512*(jc+1)],
                                                 start=(c == 0), stop=(c == 7))
                        def vevac(g=g, cell=cell):
                            for jc in range(2):
                                dst = VA[g][:, 520*jc:520*(jc+1)].rearrange(
                                    "p (h d) -> p h d", d=D+1)[:, :, 0:D]
                                src = cell["pv"][jc][:].rearrange("p (h d) -> p h d", d=D)
                                nc.vector.tensor_copy(out=dst, in_=src)
                        for c in range(8):
                            if c == 0:
                                steps.append(lambda c=c, a=valloc, v=vstep: (a(), v(c)))
                            else:
                                steps.append(lambda c=c, v=vstep: v(c))
                        steps.append(vevac)
                    return steps

                def proj_steps():
                    steps = []
                    for gi in range(12):
                        ti, jc = gi // 2, gi % 2
                        s_, half = ti // 2, ti % 2
                        cell = {}
                        def palloc(gi=gi, cell=cell):
                            cell["py"] = kvps.tile([128, 512], F32, name=f"kv{gi % 2}",
                                                   tag=f"kv{gi % 2}")
                        def pstep(c, s_=s_, half=half, jc=jc, cell=cell):
                            nc.tensor.matmul(out=cell["py"][:],
                                             lhsT=OT[c][s_][:, 128*half:128*(half+1)],
                                             rhs=wpt[c][:, 512*jc:512*(jc+1)],
                                             start=(c == 0), stop=(c == 7))
                        def pevac(ti=ti, jc=jc, cell=cell):
                            ysb = yp.tile([128, 512], F32, name="ysb", tag="ysb")
                            nc.vector.scalar_tensor_tensor(out=ysb[:], in0=cell["py"][:],
                                                           scalar=0.0,
                                                           in1=bpeb[:, 512*jc:512*(jc+1)],
                                                           op0=bypass, op1=add)
                            nc.sync.dma_start(out=y_d[128*ti:128*(ti+1), 512*jc:512*(jc+1)],
                                              in_=ysb[:])
                        for c in range(8):
                            if c == 0:
                                steps.append(lambda c=c, a=palloc, p=pstep: (a(), p(c)))
                            else:
                                steps.append(lambda c=c, p=pstep: p(c))
                        steps.append(pevac)
                    return steps

                def proj_tail(gi):
                    ti, jc = gi // 2, gi % 2
                    s_, half = ti // 2, ti % 2
                    py = kvps.tile([128, 512], F32, name=f"kv{gi % 2}", tag=f"kv{gi % 2}")
                    for c in range(8):
                        nc.tensor.matmul(out=py[:],
                                         lhsT=OT[c][s_][:, 128*half:128*(half+1)],
                                         rhs=wpt[c][:, 512*jc:512*(jc+1)],
                                         start=(c == 0), stop=(c == 7))
                    ysb = yp.tile([128, 512], F32, name="ysb", tag="ysb")
                    nc.vector.scalar_tensor_tensor(out=ysb[:], in0=py[:], scalar=0.0,
                                                   in1=bpeb[:, 512*jc:512*(jc+1)],
                                                   op0=bypass, op1=add)
                    nc.sync.dma_start(out=y_d[128*ti:128*(ti+1), 512*jc:512*(jc+1)], in_=ysb[:])

                # ---- slab 0 dense (before any attention) ----
                for st in slab_steps(xts0, 0):
                    st()

                # ---- slots with fine-grained filler interleave ----
                for s in range(NSLOT):
                    E = EXT[s]
                    if s < 3:
                        nxts = load_xt(s + 1)
                        steps = slab_steps(nxts, s + 1)
                    else:
                        steps = proj_steps()
                    n_g = 8 * (E // 2)
                    fi = 0
                    gcount = 0
                    for j in range(8):
                        o2 = aps.tile([65, 512], F32, name="o2", tag="o2", bufs=2)
                        for g in range(E // 2):
                            masked = (2 * g) >= E - 4
                            ss = aps.tile([128, 1024], F32, name="ss", tag="ss", bufs=2)
                            for u in range(2):
                                m = 2 * g + u
                                sl, mm = m // 4, m % 4
                                if masked:
                                    mi = m - (E - 4)
                                    for h in range(2):
                                        nc.tensor.matmul(out=ss[:, 512*h+QC*u:512*h+QC*(u+1)],
                                                         lhsT=ID[:], rhs=MK[s][mi][:],
                                                         start=True, stop=False,
                                                         skip_group_check=True)
                                for h in range(2):
                                    nc.tensor.matmul(
                                        out=ss[:, 512*h+QC*u:512*h+QC*(u+1)],
                                        lhsT=KT[j][sl][64*h:64*(h+1), 128*mm:128*(mm+1)],
                                        rhs=QT[j][s // 2][64*h:64*(h+1), QC*(s % 2):QC*(s % 2 + 1)],
                                        tile_position=(64 * h, 0),
                                        start=(not masked), stop=True,
                                        skip_group_check=masked)
                            pt = ptp.tile([128, 1024], BF16, name="pt", tag="pt")
                            nc.scalar.activation(out=pt[:], in_=ss[:], func=EXP)
                            for u in range(2):
                                m = 2 * g + u
                                for h in range(2):
                                    nc.tensor.matmul(out=o2[:, QC*h:QC*(h+1)],
                                                     lhsT=VA[m][:, 65*(2*j+h):65*(2*j+h)+65],
                                                     rhs=pt[:, 512*h+QC*u:512*h+QC*(u+1)],
                                                     start=(m == 0 and h == 0),
                                                     stop=(m == E - 1),
                                                     skip_group_check=True)
                            gcount += 1
                            target = (len(steps) * gcount) // n_g
                            while fi < target:
                                steps[fi]()
                                fi += 1
                        lsb = smp.tile([1, 512], F32, name="lsb", tag="lsb")
                        nc.vector.tensor_copy(out=lsb[:], in_=o2[64:65, :])
                        rsb = smp.tile([1, 512], F32, name="rsb", tag="rsb")
                        nc.vector.reciprocal_approx_fast(rsb[:], lsb[:])
                        rbb = smp.tile([64, 512], F32, name="rbb", tag="rbb")
                        nc.gpsimd.partition_broadcast(rbb[:], rsb[:])
                        for h in range(2):
                            nc.vector.scalar_tensor_tensor(
                                out=OT[j][s][64*h:64*(h+1), :], in0=o2[0:64, QC*h:QC*(h+1)],
                                scalar=0.0, in1=rbb[:, QC*h:QC*(h+1)],
                                op0=bypass, op1=mult)
                    while fi < len(steps):
                        steps[fi]()
                        fi += 1
                for gi in range(12, 16):
                    proj_tail(gi)
        pers.release()


    nc.compile()
    return nc


def _get_nc():
    if "nc" not in _cache:
        _cache["nc"] = _build()
    return _cache["nc"]


def _host_prep(x, Wqkv, bqkv, Wproj, bproj):
    bf = ml_dtypes.bfloat16
    x = np.ascontiguousarray(np.asarray(x, dtype=np.float32))
    Wqkv = np.asarray(Wqkv, dtype=np.float32)
    bqkv = np.asarray(bqkv, dtype=np.float32)
    Wproj = np.ascontiguousarray(np.asarray(Wproj, dtype=np.float32))
    bproj = np.asarray(bproj, dtype=np.float32)

    wq = np.ascontiguousarray(Wqkv[:, :C] * np.float32(0.125)).astype(bf)
    wk = np.ascontiguousarray(Wqkv[:, C:2*C]).astype(bf)
    wv = np.ascontiguousarray(Wqkv[:, 2*C:]).astype(bf)
    wp = Wproj.astype(bf)
    bq8 = (bqkv[:C] * np.float32(0.125)).reshape(8, 128, 1).copy()
    bk8 = bqkv[C:2*C].reshape(8, 128, 1).copy()
    bv = bqkv[2*C:]
    bpe = (bproj.astype(np.float64) + bv.astype(np.float64) @ Wproj.astype(np.float64)).astype(np.float32)
    bpeb = np.ascontiguousarray(np.broadcast_to(bpe, (128, C)))
    id128 = np.eye(128, dtype=bf)

    pidx = np.arange(128)[:, None]
    fidx = np.arange(QC)[None, :]
    masks = []
    for par in range(2):
        mk = np.zeros((NSLOT, 4, 128, QC), dtype=np.float32)
        for s, cchunk in enumerate(OWN[par]):
            for mi in range(4):
                g = EXT[s] - 4 + mi
                mk[s, mi] = np.where((128*g + pidx) <= (QC*cchunk + fidx), 0.0, NEG)
        masks.append(mk.astype(bf))

    in_maps = []
    for core in range(8):
        b, par = core // 2, core % 2
        xt = np.ascontiguousarray(x[b].T)
        xq = np.ascontiguousarray(
            np.concatenate([xt[:, QC*c:QC*(c+1)] for c in OWN[par]], axis=1)).astype(bf)
        in_maps.append(dict(xt=xt.astype(bf), xq=xq, wq=wq, wk=wk, wv=wv, wp=wp,
                            bq=bq8, bk=bk8, bpeb=bpeb, masks=masks[par],
                            id128=id128))
    return in_maps


def kernel(x, Wqkv, bqkv, Wproj, bproj):
    nc = _get_nc()
    in_maps = _host_prep(x, Wqkv, bqkv, Wproj, bproj)
    trace = bool(os.environ.get("BASS_TRACE"))
    res = run_bass_kernel_spmd(nc, in_maps, list(range(8)), trace=trace)
    _cache["last_exec_time_ns"] = res.exec_time_ns
    _cache["last_res"] = res
    out = np.empty((B, T, C), dtype=np.float32)
    for core in range(8):
        b, par = core // 2, core % 2
        y = res.results[core]["y"]
        for s, cchunk in enumerate(OWN[par]):
            out[b, QC*cchunk:QC*(cchunk+1)] = y[QC*s:QC*(s+1)]
    return out
